# revision 1
# baseline (speedup 1.0000x reference)
"""ColumnParallelLinear kernel for Trainium2 (8 NeuronCores).

Computes Y[s,b,o] = sum_h X[s,b,h] * W[o,h]  (F.linear / einsum 'sbh,oh->sbo')
with S,B,H,OUT = 2048,4,1024,4096, fp32 in/out.

Strategy:
  - Flatten tokens: M = S*B = 8192 rows.  GEMM: [M,H] @ [H,OUT].
  - 2D shard over 8 cores: 4 token groups (2048 rows) x 2 out-column
    groups (2048 cols) -- minimizes per-core HBM traffic.
  - Inputs cast to bf16 on host (rel err ~3e-3, full-rate PE path).
  - DMA queues are packet-rate limited (~28-36ns per packet, one packet
    per partition per dma_start), so loads are whole 1MB chunks (128
    packets each, 8KB/partition) scheduled in queue start order:
    sync starts earliest and carries w0/x0.
  - Warmup matmuls on a memset tile keep the PE busy (and its clock
    ramped to 2.4GHz) while the first real tiles arrive.
  - Matmuls: 128x128 bf16 stationary (x), 512-wide moving (w),
    accumulating fp32 into [128,512] PSUM tiles (all 8 banks rotate;
    warmup shares the pool slot).  PSUM -> SBUF stage rows via vector
    copies; full 8KB-run row writes round-robin over the queues; the
    final row's writes are partition-split across all three queues to
    shorten the drain after the last matmul.
"""

import numpy as np
import ml_dtypes

import concourse.bass as bass
from concourse import bacc
import concourse.mybir as mybir
import concourse.tile as tile
from concourse.bass_utils import run_bass_kernel_spmd

S, B, H, OUT = 2048, 4, 1024, 4096
M = S * B

N_CORES = 8
G_ROW, G_COL = 4, 2          # token groups x out-feature groups
M_LOC = M // G_ROW           # 2048 rows per core
N_LOC = OUT // G_COL         # 2048 out features per core

P = 128
KO = H // P                  # 8 contraction subtiles
NT = 512                     # one n-chunk (DMA unit)
NO = N_LOC // NT             # 4 n-chunks
NW = 512                     # matmul moving width (one psum bank)
NH = N_LOC // NW             # 4 col tiles
XG = 512                     # x chunk width (4 row tiles)
NXG = M_LOC // XG            # 4 chunks
MO = M_LOC // P              # 16 row tiles

MM_DT = mybir.dt.bfloat16
N_WARM = 38                  # warmup matmuls during initial DMA wait


def build_nc(mm_dt=MM_DT):
    nc = bacc.Bacc(None, target_bir_lowering=False, enable_partition_id=False)
    xH = nc.declare_dram_parameter("xH", [NXG, P, KO, XG], mm_dt,
                                   isOutput=False)
    wH = nc.declare_dram_parameter("wH", [NO, P, KO, NT], mm_dt,
                                   isOutput=False)
    y = nc.declare_dram_parameter("y", [M_LOC, N_LOC], mybir.dt.float32,
                                  isOutput=True)
    y_r = y[:, :].rearrange("(mo p) n -> p mo n", p=P)

    with tile.TileContext(nc) as tc:
        with (
            tc.tile_pool(name="xp", bufs=1) as xp,
            tc.tile_pool(name="wp", bufs=1) as wp,
            tc.tile_pool(name="op", bufs=4) as op,
            tc.tile_pool(name="psp", bufs=8, space="PSUM") as psp,
        ):
            def alloc_ps():
                # single tag so warmup + main loop share the pool slot
                return psp.tile([P, NW], mybir.dt.float32, tag="ps",
                                name="ps")

            # ---- PE warmup: matmuls on a zeroed tile, no DMA deps ----
            warm = xp.tile([P, 128 + NT], mm_dt, tag="warm", name="warm")
            nc.vector.memset(warm[:], 0.0)
            for i in range(N_WARM):
                wps = alloc_ps()
                nc.tensor.matmul(wps[:, 0:NT], lhsT=warm[:, :128],
                                 rhs=warm[:, 128:128 + NT],
                                 start=True, stop=True)

            # ---- input tiles ----
            x_sb = [xp.tile([P, KO, XG], mm_dt, tag=f"x{g}", name=f"x{g}")
                    for g in range(NXG)]
            w_sb = wp.tile([P, NO, KO, NT], mm_dt, tag="w", name="w")

            # startup-critical set is w0 + x0: one whole 128-packet chunk
            # first on each fast queue (sync/scalar start order is random
            # run-to-run; gpsimd always starts late so it gets no loads)
            nc.sync.dma_start(w_sb[:, 0, :, :], wH[0, :, :, :])
            nc.scalar.dma_start(x_sb[0][:], xH[0, :, :, :])
            nc.scalar.dma_start(w_sb[:, 1, :, :], wH[1, :, :, :])
            nc.sync.dma_start(x_sb[1][:], xH[1, :, :, :])
            nc.sync.dma_start(w_sb[:, 2, :, :], wH[2, :, :, :])
            nc.scalar.dma_start(x_sb[2][:], xH[2, :, :, :])
            nc.scalar.dma_start(w_sb[:, 3, :, :], wH[3, :, :, :])
            nc.sync.dma_start(x_sb[3][:], xH[3, :, :, :])

            QUEUES = [nc.sync, nc.scalar, nc.gpsimd]
            rr = [0]  # round-robin cursor for y writes

            def write_row(mo, stage, allow_gpsimd=True):
                nq = 3 if allow_gpsimd else 2
                q = QUEUES[rr[0] % nq]
                rr[0] += 1
                q.dma_start(y_r[:, mo, :], stage[:])

            def do_group(g, nh_outer, tail=False):
                stages = [op.tile([P, N_LOC], mybir.dt.float32, tag=f"st{mi}",
                                  name=f"st{g}_{mi}")
                          for mi in range(XG // P)]
                outer = range(NH) if nh_outer else range(XG // P)
                inner = range(XG // P) if nh_outer else range(NH)
                last = XG // P - (1 if tail else 0)
                for a in outer:
                    for b in inner:
                        nh, mi = (a, b) if nh_outer else (b, a)
                        ps = alloc_ps()
                        last_block = tail and mi == XG // P - 1 and nh == NH - 1
                        for k in range(KO):
                            nc.tensor.matmul(
                                ps[:],
                                lhsT=x_sb[g][:, k, mi * P:(mi + 1) * P],
                                rhs=w_sb[:, nh, k, 0:NT],
                                start=(k == 0),
                                stop=(k == KO - 1),
                            )
                        if last_block:
                            # split the final copy across two engines; the
                            # very last write uses only the two fast queues
                            # (gpsimd's share was front-loaded onto earlier
                            # slices of this row)
                            HP = P // 2
                            nc.vector.tensor_copy(
                                stages[mi][0:HP, nh * NW:(nh + 1) * NW],
                                ps[0:HP, :])
                            nc.scalar.copy(
                                stages[mi][HP:P, nh * NW:(nh + 1) * NW],
                                ps[HP:P, :])
                            mo = g * (XG // P) + mi
                            bounds = [(0, 64), (64, 128)]
                            for (lo, hi), q in zip(bounds, QUEUES[:2]):
                                q.dma_start(
                                    y_r[lo:hi, mo, nh * NW:(nh + 1) * NW],
                                    stages[mi][lo:hi, nh * NW:(nh + 1) * NW],
                                )
                            continue
                        nc.vector.tensor_copy(
                            stages[mi][:, nh * NW:(nh + 1) * NW], ps[:]
                        )
                        if tail and mi == XG // P - 1:
                            # per-slice writes as copies land; gpsimd's share
                            # tapers (64/48/16) so it finishes early and the
                            # final slice rides the fast queues alone
                            mo = g * (XG // P) + mi
                            taper = {0: 64, 1: 48, 2: 16}[nh]
                            fast = (P - taper) // 2
                            bounds = [(0, fast), (fast, 2 * fast),
                                      (2 * fast, P)]
                            for (lo, hi), q in zip(bounds, QUEUES):
                                q.dma_start(
                                    y_r[lo:hi, mo, nh * NW:(nh + 1) * NW],
                                    stages[mi][lo:hi, nh * NW:(nh + 1) * NW],
                                )
                        # issue each full row write as soon as its stage
                        # completes (all nh copies done) instead of in a
                        # burst at group end: smoother queues, earlier WAR
                        # clearance for the next group's stage reuse
                        row_done = (nh == NH - 1) if not nh_outer else \
                                   (a == NH - 1)
                        if row_done and mi < last:
                            mo = g * (XG // P) + mi
                            # keep the slow gpsimd queue off the late writes
                            # so it never gates the drain
                            write_row(mo, stages[mi], allow_gpsimd=not tail)

            do_group(0, nh_outer=True)    # w arrives n-chunk by n-chunk
            for g in range(1, NXG):
                # mi-outer spreads the writes
                do_group(g, nh_outer=False, tail=(g == NXG - 1))

    nc.compile()
    return nc


def make_in_maps(input_, weight):
    X = np.asarray(input_, dtype=np.float32).reshape(M, H)
    W = np.asarray(weight, dtype=np.float32)
    in_maps = []
    for c in range(N_CORES):
        i, j = divmod(c, G_COL)
        # xH[g, p, k, mg] = X[i*M_LOC + g*XG + mg, k*P + p]
        xc = X[i * M_LOC:(i + 1) * M_LOC]                  # [M_LOC, H]
        xh = np.ascontiguousarray(
            xc.reshape(NXG, XG, KO, P).transpose(0, 3, 2, 1)
        ).astype(ml_dtypes.bfloat16)
        # wH[n, p, k, nq] = W[j*N_LOC + n*NT + nq, k*P + p]
        wc = W[j * N_LOC:(j + 1) * N_LOC]                  # [N_LOC, H]
        wh = np.ascontiguousarray(
            wc.reshape(NO, NT, KO, P).transpose(0, 3, 2, 1)
        ).astype(ml_dtypes.bfloat16)
        in_maps.append({"xH": xh, "wH": wh})
    return in_maps


def assemble(results):
    Y = np.empty((M, OUT), dtype=np.float32)
    for c in range(N_CORES):
        i, j = divmod(c, G_COL)
        Y[i * M_LOC:(i + 1) * M_LOC, j * N_LOC:(j + 1) * N_LOC] = results[c]["y"]
    return Y.reshape(S, B, OUT)


def kernel(input_, weight):
    nc = build_nc()
    res = run_bass_kernel_spmd(nc, make_in_maps(input_, weight), list(range(N_CORES)))
    return assemble(res.results)

